# revision 1
# baseline (speedup 1.0000x reference)
"""Trainium2 Bass kernel for nn_ConvertParamsLayerFrom0_16277926052103.

Math (per batch b):
    inv[b,h]  = 1/sqrt(covh_diag2[b,h])
    wt2[b,h,v] = wt1[b,h,v] * inv[b,h]
    b2[b,v]    = b1[b,v] - sum_h wt1[b,h,v] * inv[b,h] * muh2[b,h]

Sharding: pure data-parallel over batch dim B=2048 across 8 NeuronCores
(256 batches/core), no cross-core communication.

Per-core dataflow (2 groups of 128 batches):
  - covh/muh/b1 group slabs load batch-major [128b x 128h]; inv = sqrt(1/x)
    (DVE reciprocal + ACT Sqrt), s = inv*muh; TensorE-transpose inv,s to
    [128h x 128b] so per-batch scales are per-partition columns.
  - wt1 sub-blocks of TB batches load as [128h x TB*256v] (1KB contiguous
    chunks, 4MiB per DMA).  Per batch j: DVE tensor_scalar_mul broadcasts
    inv_T[:,j] along the free dim -> wt2 tile; TensorE matmuls
    lhsT=wt1[:,j,vhalf] (stationary), rhs=s_T[:,j] -> psum column
    r[vhalf, j] = sum_h wt1[h,v]*s[h].
  - After a group: copy psum r-halves to SBUF, TensorE-transpose back to
    batch-major, b2 = b1 - r, DMA out.
"""

import os
import sys

import numpy as np

B, NH, NV = 2048, 128, 256
NCORES = 8
BL = B // NCORES          # 256 batches per core
GP = 128                  # batches per group (one transpose block)
GROUPS = BL // GP         # 2
TB = 32                   # batches per wt1/wt2 DMA sub-block
NSUB = GP // TB           # 4
NVH = NV // 2             # 128 (matmul stationary free-dim limit)

_cache: dict = {}


def _ensure_import_paths():
    try:
        import concourse  # noqa: F401
    except ImportError:
        for p in ("/root/.axon_site", "/root/.axon_site/_ro/trn_rl_repo",
                  "/root/.axon_site/_ro/pypackages", "/opt/trn_rl_repo"):
            if os.path.isdir(p) and p not in sys.path:
                sys.path.append(p)


def _build():
    """Build + compile the per-core Bass program (cached)."""
    if "nc" in _cache:
        return _cache["nc"]

    _ensure_import_paths()
    import concourse.bacc as bacc
    import concourse.bass as bass
    import concourse.mybir as mybir
    import concourse.tile as tile
    from concourse.masks import make_identity

    f32 = mybir.dt.float32

    nc = bacc.Bacc("TRN2", target_bir_lowering=False, debug=False,
                   enable_asserts=False)

    b1_d = nc.dram_tensor("b1", [BL, NV], f32, kind="ExternalInput")
    wt1_d = nc.dram_tensor("wt1", [BL, NH, NV], f32, kind="ExternalInput")
    muh_d = nc.dram_tensor("muh2", [BL, NH], f32, kind="ExternalInput")
    covh_d = nc.dram_tensor("covh_diag2", [BL, NH], f32, kind="ExternalInput")
    b2_d = nc.dram_tensor("b2", [BL, NV], f32, kind="ExternalOutput")
    wt2_d = nc.dram_tensor("wt2", [BL, NH, NV], f32, kind="ExternalOutput")

    with tile.TileContext(nc) as tc:
        with (
            tc.tile_pool(name="consts", bufs=1) as consts,
            tc.tile_pool(name="small", bufs=2) as small,
            tc.tile_pool(name="w_in", bufs=2) as w_in,
            tc.tile_pool(name="w_out", bufs=2) as w_out,
            tc.tile_pool(name="ps_t", bufs=4, space=bass.MemorySpace.PSUM) as ps_t,
            tc.tile_pool(name="ps_r", bufs=4, space=bass.MemorySpace.PSUM) as ps_r,
        ):
            identity = consts.tile([128, 128], f32)
            make_identity(nc, identity[:])

            for g in range(GROUPS):
                b0 = g * GP

                covh_t = small.tile([GP, NH], f32)
                muh_t = small.tile([GP, NH], f32)
                b1_t = small.tile([GP, NV], f32)
                nc.sync.dma_start(covh_t[:], covh_d[b0:b0 + GP])
                nc.sync.dma_start(muh_t[:], muh_d[b0:b0 + GP])
                nc.sync.dma_start(b1_t[:], b1_d[b0:b0 + GP])

                # inv = sqrt(1/covh)   (Rsqrt activation is banned for accuracy)
                inv = small.tile([GP, NH], f32)
                nc.vector.reciprocal(inv[:], covh_t[:])
                nc.scalar.activation(inv[:], inv[:],
                                     mybir.ActivationFunctionType.Sqrt)
                s = small.tile([GP, NH], f32)
                nc.vector.tensor_mul(s[:], inv[:], muh_t[:])

                # transpose scales to [h, batch] so batches are columns
                inv_ps = ps_t.tile([NH, GP], f32, tag="tp")
                nc.tensor.transpose(inv_ps[:], inv[:], identity[:])
                invT = small.tile([NH, GP], f32)
                nc.vector.tensor_copy(invT[:], inv_ps[:])

                s_ps = ps_t.tile([NH, GP], f32, tag="tp")
                nc.tensor.transpose(s_ps[:], s[:], identity[:])
                sT = small.tile([NH, GP], f32)
                nc.vector.tensor_copy(sT[:], s_ps[:])

                # r[v, j] = sum_h wt1[b0+j, h, v] * s[b0+j, h], v split in halves
                rA = ps_r.tile([NVH, GP], f32, tag="r")
                rB = ps_r.tile([NVH, GP], f32, tag="r")

                for k in range(NSUB):
                    sb = b0 + k * TB
                    w1t = w_in.tile([NH, TB, NV], f32)
                    nc.sync.dma_start(
                        w1t[:], wt1_d[sb:sb + TB].rearrange("b h v -> h b v"))
                    w2t = w_out.tile([NH, TB, NV], f32)
                    for j in range(TB):
                        gj = k * TB + j
                        nc.tensor.matmul(rA[:, gj:gj + 1], w1t[:, j, 0:NVH],
                                         sT[:, gj:gj + 1])
                        nc.tensor.matmul(rB[:, gj:gj + 1], w1t[:, j, NVH:NV],
                                         sT[:, gj:gj + 1])
                        nc.vector.tensor_scalar_mul(w2t[:, j, :], w1t[:, j, :],
                                                    invT[:, gj:gj + 1])
                    nc.sync.dma_start(
                        wt2_d[sb:sb + TB].rearrange("b h v -> h b v"), w2t[:])

                # back to batch-major and subtract
                rA_sb = small.tile([NVH, GP], f32)
                nc.vector.tensor_copy(rA_sb[:], rA[:])
                rB_sb = small.tile([NVH, GP], f32)
                nc.vector.tensor_copy(rB_sb[:], rB[:])
                rAT = ps_t.tile([GP, NVH], f32, tag="tp")
                nc.tensor.transpose(rAT[:], rA_sb[:], identity[:])
                rBT = ps_t.tile([GP, NVH], f32, tag="tp")
                nc.tensor.transpose(rBT[:], rB_sb[:], identity[:])

                b2_t = small.tile([GP, NV], f32)
                nc.vector.tensor_sub(b2_t[:, 0:NVH], b1_t[:, 0:NVH], rAT[:])
                nc.vector.tensor_sub(b2_t[:, NVH:NV], b1_t[:, NVH:NV], rBT[:])
                nc.sync.dma_start(b2_d[b0:b0 + GP], b2_t[:])

    nc.compile()
    _cache["nc"] = nc
    return nc


def kernel(b1, wt1, muh2, covh_diag2):
    _ensure_import_paths()
    from concourse.bass_utils import run_bass_kernel_spmd

    nc = _build()

    b1 = np.ascontiguousarray(np.asarray(b1, dtype=np.float32))
    wt1 = np.ascontiguousarray(np.asarray(wt1, dtype=np.float32))
    muh2 = np.ascontiguousarray(np.asarray(muh2, dtype=np.float32))
    covh = np.ascontiguousarray(np.asarray(covh_diag2, dtype=np.float32))

    in_maps = []
    for c in range(NCORES):
        lo, hi = c * BL, (c + 1) * BL
        in_maps.append({
            "b1": b1[lo:hi],
            "wt1": wt1[lo:hi],
            "muh2": muh2[lo:hi],
            "covh_diag2": covh[lo:hi],
        })

    res = run_bass_kernel_spmd(nc, in_maps, list(range(NCORES))).results
    b2 = np.concatenate([r["b2"] for r in res], axis=0)
    wt2 = np.concatenate([r["wt2"] for r in res], axis=0)
    return b2, wt2


# revision 2
# speedup vs baseline: 1.1510x; 1.1510x over previous
"""Trainium2 Bass kernel for nn_ConvertParamsLayerFrom0_16277926052103.

Layout: sub-blocks of SB=32 batches; SBUF partition p = 4*b_local + q
(q = h//32 quarter), so each partition's slice of wt1 is one contiguous
32KB HBM range -> near-peak DMA efficiency (vs 1KB chunks in v1).

Matvec r[b,v] = sum_h wt1[b,h,v]*s[b,h]: for each h-chunk hc (h%32),
matmul with stationary L[:,hc,:] = E * s2[:,hc] where E[p,m] =
(p//4 == m) expands batches to partition groups; rhs = W[:,hc,:].
PSUM accumulates over the 32 chunks into a batch-major [32,256] tile —
no transposes, no per-batch weight reloads.

wt2 scaling: per h-chunk tensor_scalar with per-partition scalar
inv2[:,hc], alternating DVE/ACT to split the elementwise load.
"""

import os
import sys

import numpy as np

B, NH, NV = 2048, 128, 256
NCORES = 8
BL = B // NCORES          # 256 batches per core
SB = 32                   # batches per sub-block
NSUB = BL // SB           # 8
XQ = 4                    # h-quarters per batch -> 4*32 = 128 partitions
HC = NH // XQ             # 32 h-chunks per quarter (free-dim h index)

_cache: dict = {}


def _ensure_import_paths():
    try:
        import concourse  # noqa: F401
    except ImportError:
        for p in ("/root/.axon_site", "/root/.axon_site/_ro/trn_rl_repo",
                  "/root/.axon_site/_ro/pypackages", "/opt/trn_rl_repo"):
            if os.path.isdir(p) and p not in sys.path:
                sys.path.append(p)


def _build():
    if "nc" in _cache:
        return _cache["nc"]

    _ensure_import_paths()
    import concourse.bacc as bacc
    import concourse.bass as bass
    import concourse.mybir as mybir
    import concourse.tile as tile

    f32 = mybir.dt.float32

    nc = bacc.Bacc("TRN2", target_bir_lowering=False, debug=False,
                   enable_asserts=False)

    b1_d = nc.dram_tensor("b1", [BL, NV], f32, kind="ExternalInput")
    wt1_d = nc.dram_tensor("wt1", [BL, NH, NV], f32, kind="ExternalInput")
    muh_d = nc.dram_tensor("muh2", [BL, NH], f32, kind="ExternalInput")
    covh_d = nc.dram_tensor("covh_diag2", [BL, NH], f32, kind="ExternalInput")
    b2_d = nc.dram_tensor("b2", [BL, NV], f32, kind="ExternalOutput")
    wt2_d = nc.dram_tensor("wt2", [BL, NH, NV], f32, kind="ExternalOutput")

    # E[p, m] = 1 if p//XQ == m else 0  (batch -> partition-group expansion)
    e_np = np.kron(np.eye(SB, dtype=np.float32),
                   np.ones((XQ, 1), dtype=np.float32))
    e_d = nc.inline_tensor(e_np, name="Emat")

    with tile.TileContext(nc) as tc:
        with (
            tc.tile_pool(name="consts", bufs=1) as consts,
            tc.tile_pool(name="small", bufs=3) as small,
            tc.tile_pool(name="w_in", bufs=2) as w_in,
            tc.tile_pool(name="w_out", bufs=2) as w_out,
            tc.tile_pool(name="ps_r", bufs=4, space=bass.MemorySpace.PSUM) as ps_r,
        ):
            E = consts.tile([XQ * SB, SB], f32)
            nc.sync.dma_start(E[:], e_d[:])

            for k in range(NSUB):
                sb = k * SB

                W = w_in.tile([128, HC, NV], f32)
                nc.sync.dma_start(
                    W[:], wt1_d[sb:sb + SB].rearrange(
                        "b (x h) v -> (b x) h v", x=XQ))

                covh2 = small.tile([128, HC], f32)
                nc.sync.dma_start(
                    covh2[:], covh_d[sb:sb + SB].rearrange(
                        "b (x h) -> (b x) h", x=XQ))
                muh2t = small.tile([128, HC], f32)
                nc.sync.dma_start(
                    muh2t[:], muh_d[sb:sb + SB].rearrange(
                        "b (x h) -> (b x) h", x=XQ))
                b1t = small.tile([SB, NV], f32)
                nc.sync.dma_start(b1t[:], b1_d[sb:sb + SB])

                inv2 = small.tile([128, HC], f32)
                nc.vector.reciprocal(inv2[:], covh2[:])
                nc.scalar.activation(inv2[:], inv2[:],
                                     mybir.ActivationFunctionType.Sqrt)
                s2 = small.tile([128, HC], f32)
                nc.vector.tensor_mul(s2[:], inv2[:], muh2t[:])

                # L[p, hc, m] = E[p, m] * s2[p, hc]
                L = small.tile([128, HC, SB], f32)
                nc.vector.tensor_mul(
                    L[:], E[:, None, :].broadcast_to([128, HC, SB]),
                    s2[:, :, None].broadcast_to([128, HC, SB]))

                R = ps_r.tile([SB, NV], f32, tag="r")
                for hc in range(HC):
                    nc.tensor.matmul(R[:], L[:, hc, :], W[:, hc, :],
                                     start=(hc == 0), stop=(hc == HC - 1))

                W2 = w_out.tile([128, HC, NV], f32)
                for hc in range(HC):
                    if hc % 2 == 0:
                        nc.vector.tensor_scalar_mul(
                            W2[:, hc, :], W[:, hc, :], inv2[:, hc:hc + 1])
                    else:
                        nc.scalar.activation(
                            W2[:, hc, :], W[:, hc, :],
                            mybir.ActivationFunctionType.Copy,
                            scale=inv2[:, hc:hc + 1])
                nc.sync.dma_start(
                    wt2_d[sb:sb + SB].rearrange("b (x h) v -> (b x) h v",
                                                x=XQ), W2[:])

                b2t = small.tile([SB, NV], f32)
                nc.vector.tensor_sub(b2t[:], b1t[:], R[:])
                nc.sync.dma_start(b2_d[sb:sb + SB], b2t[:])

    nc.compile()
    _cache["nc"] = nc
    return nc


def kernel(b1, wt1, muh2, covh_diag2):
    _ensure_import_paths()
    from concourse.bass_utils import run_bass_kernel_spmd

    nc = _build()

    b1 = np.ascontiguousarray(np.asarray(b1, dtype=np.float32))
    wt1 = np.ascontiguousarray(np.asarray(wt1, dtype=np.float32))
    muh2 = np.ascontiguousarray(np.asarray(muh2, dtype=np.float32))
    covh = np.ascontiguousarray(np.asarray(covh_diag2, dtype=np.float32))

    in_maps = []
    for c in range(NCORES):
        lo, hi = c * BL, (c + 1) * BL
        in_maps.append({
            "b1": b1[lo:hi],
            "wt1": wt1[lo:hi],
            "muh2": muh2[lo:hi],
            "covh_diag2": covh[lo:hi],
        })

    res = run_bass_kernel_spmd(nc, in_maps, list(range(NCORES))).results
    b2 = np.concatenate([r["b2"] for r in res], axis=0)
    wt2 = np.concatenate([r["wt2"] for r in res], axis=0)
    return b2, wt2
